# revision 30
# baseline (speedup 1.0000x reference)
"""Trainium2 Bass kernel for AdaptiveStochasticSNN.

Model: x[B,T,D] -> FC1(D->H) -> StochasticAdaptiveLIF -> FC2(H->A)
       -> StochasticAdaptiveLIF -> mean spikes over T.   B,T,D,H,A = 256,64,6400,1000,4

Strategy (8 NeuronCores, data-parallel over batch, 32 batches/core):
- FC1 hoisted out of the time loop as one big GEMM x[bt,D] @ W1T[D,H] in fp16
  (full PE rate; fp16's 11-bit significand rounds identically to f32r/TF32 for
  this data range). fp16 storage halves HBM traffic and lets all of W1T
  (13.1MB) stay SBUF-resident, so the kernel is TensorE-bound, not DMA-bound.
- The bernoulli draw  u < sigmoid(mem - 1 - theta)  is monotone-transformed on
  the host to  logit(u) + 1 < mem - theta  (plain DVE compare, no sigmoid).
- LIF1 runs as fused scalar_tensor_tensor ops on VectorE with h on partitions;
  FC2 consumes the spike complement ge (= 1 - spk): cur2 = (rowsum(W2)+b2)
  - W2 @ ge, as [A, bt] so LIF2 runs on partitions 0..3 with no shuffle.
- 5 tapered windows of [16,16,16,12,4] timesteps pipeline GEMM(w) over
  LIF1(w-1) and FC2/LIF2(w-2). The taper keeps the serial DVE tail short.
- Two HWDGE DMA rings: the sync ring streams x chunks; the scalar ring takes
  the W1-resident load, per-window logit-u tiles, and constants.
"""

import sys

sys.path.insert(0, "/opt/trn_rl_repo")

import numpy as np

# ---- problem dims (hardcoded; kernel.py must be self-contained) ----
B, T, D, H, A = 256, 64, 6400, 1000, 4
HP = 1024          # H padded to 8*128
NCORES = 8
BC = B // NCORES   # 32 batches per core
BT = BC * T        # 2048 bt-columns per core, ordered bt = t*BC + b
KC = D // 128      # 50 contraction chunks
MC = HP // 128     # 8 h-chunks
WINS = [16, 16, 16, 12, 4]   # timesteps per window (sum = T)
NW = len(WINS)
OFF = [sum(WINS[:i]) for i in range(NW)]
BETA = 0.9
TH_DEC = 0.9
TH_PLUS = 0.05
LUPAD = np.float16(60000.0)  # threshold for padded h lanes: never spike

_CACHE = {}


def _build_graph():
    import concourse.bass as bass
    import concourse.tile as tile
    from concourse import bacc, mybir
    from concourse.alu_op_type import AluOpType as op
    from contextlib import ExitStack

    F32 = mybir.dt.float32
    F16 = mybir.dt.float16
    AF = mybir.ActivationFunctionType

    nc = bacc.Bacc("TRN2", target_bir_lowering=False, debug=False, num_devices=NCORES)

    xt = nc.declare_dram_parameter("xt", [D, BT], F16, isOutput=False)
    w1t = nc.declare_dram_parameter("w1t", [D, HP], F16, isOutput=False)
    b1t = nc.declare_dram_parameter("b1t", [128, MC], F32, isOutput=False)
    lu1 = nc.declare_dram_parameter("lu1", [128, MC, BT], F16, isOutput=False)
    lu2 = nc.declare_dram_parameter("lu2", [A, T, BC], F16, isOutput=False)
    w2t = nc.declare_dram_parameter("w2t", [128, MC, A], F16, isOutput=False)
    rs2b = nc.declare_dram_parameter("rs2b", [A, 512], F32, isOutput=False)
    out = nc.declare_dram_parameter("out", [A, BC], F32, isOutput=True)

    with tile.TileContext(nc) as tc, ExitStack() as ctx:
        p_w1 = ctx.enter_context(tc.tile_pool(name="w1p", bufs=1))
        p_x = ctx.enter_context(tc.tile_pool(name="xp", bufs=6))
        p_lu = ctx.enter_context(tc.tile_pool(name="lup", bufs=2))
        p_cur = ctx.enter_context(tc.tile_pool(name="curp", bufs=1))
        p_ge = ctx.enter_context(tc.tile_pool(name="gep", bufs=2))
        p_sc = ctx.enter_context(tc.tile_pool(name="scp", bufs=2))
        p_c2 = ctx.enter_context(tc.tile_pool(name="c2p", bufs=2))
        p_st = ctx.enter_context(tc.tile_pool(name="stp", bufs=1))
        p_ps = ctx.enter_context(
            tc.tile_pool(name="psp", bufs=8, space=bass.MemorySpace.PSUM)
        )

        # ---- persistent tiles ----
        b1_sb = p_st.tile([128, MC], F32, name="b1_sb")
        w2_sb = p_st.tile([128, MC, A], F16, name="w2_sb")
        rs2_sb = p_st.tile([A, 512], F32, name="rs2_sb")
        lu2_sb = p_st.tile([A, T, BC], F16, name="lu2_sb")

        # W1T fully resident: [128, KC, HP] fp16 = 100KB/partition
        w1_sb = p_w1.tile([128, KC, HP], F16, name="w1_sb")

        # theta is tracked as psi = 20*theta - 10, turning the update into a
        # single fused op  psi' = 0.9*psi - ge  (the compare absorbs the
        # affine map via a host-side +1.5 on the logit)
        mem = p_st.tile([128, MC, BC], F32, name="mem")
        nc.gpsimd.memset(mem[:], 0.0)
        psi = p_st.tile([128, MC, BC], F32, name="psi")
        nc.gpsimd.memset(psi[:], -10.0)
        mem2 = p_st.tile([A, BC], F32, name="mem2")
        nc.gpsimd.memset(mem2[:], 0.0)
        psi2 = p_st.tile([A, BC], F32, name="psi2")
        nc.gpsimd.memset(psi2[:], -10.0)
        sum2 = p_st.tile([A, BC], F32, name="sum2")
        nc.gpsimd.memset(sum2[:], 0.0)

        ge_tiles = [None] * NW
        c2_tiles = [None] * NW

        def load_consts():
            # scalar-ring so the sync ring stays dedicated to the x stream
            nc.scalar.dma_start(b1_sb[:], b1t[:])
            nc.scalar.dma_start(w2_sb[:], w2t[:])
            nc.scalar.dma_start(rs2_sb[:], rs2b[:])
            nc.scalar.dma_start(lu2_sb[:], lu2[:])

        def emit_fc2_post(w):
            """FC2 for window w as one full-width chunk, own psum tile
            (emitted right after GEMM of window w+1; ring slot is free)."""
            ntw = WINS[w] * BC
            ps2 = p_ps.tile([128, 512], F32, tag="acc", name=f"ps2_{w}")
            c2 = p_c2.tile([A, 512], F32, tag="c2", name=f"c2_{w}")
            c2_tiles[w] = c2
            ge_t = ge_tiles[w]
            for k2 in range(MC):
                nc.tensor.matmul(
                    ps2[:A, :ntw],
                    w2_sb[:, k2, :],
                    ge_t[:, k2, :ntw],
                    start=(k2 == 0),
                    stop=(k2 == MC - 1),
                )
            nc.vector.tensor_tensor(
                c2[:, :ntw], rs2_sb[:, :ntw], ps2[:A, :ntw], op.subtract
            )

        def emit_lif2_step(w, s):
            t = OFF[w] + s
            eng = nc.vector
            cur2 = c2_tiles[w][:, s * BC : (s + 1) * BC]
            m2i = p_sc.tile([A, BC], F32, tag="m2i", name=f"m2i_{t}")
            eng.scalar_tensor_tensor(
                m2i[:], mem2[:], BETA, cur2, op0=op.mult, op1=op.add
            )
            lp2 = p_sc.tile([A, BC], F32, tag="lp2", name=f"lp2_{t}")
            eng.scalar_tensor_tensor(
                lp2[:], psi2[:], TH_PLUS, lu2_sb[:, t, :], op0=op.mult, op1=op.add
            )
            ge2 = p_sc.tile([A, BC], F32, tag="ge2", name=f"ge2_{t}")
            eng.tensor_tensor(ge2[:], m2i[:], lp2[:], op.is_le)
            eng.tensor_tensor(mem2[:], m2i[:], ge2[:], op.mult)
            eng.scalar_tensor_tensor(
                psi2[:], psi2[:], TH_DEC, ge2[:], op0=op.mult, op1=op.subtract
            )
            eng.tensor_tensor(sum2[:], sum2[:], ge2[:], op.add)

        def emit_rec1_step(w, s, cur1, lu_t, ge_t):
            c_sl = cur1[:, :, s * BC : (s + 1) * BC]
            mi = p_sc.tile([128, MC, BC], F32, tag="mi", name=f"mi_{w}_{s}")
            nc.vector.scalar_tensor_tensor(
                mi[:], mem[:], BETA, c_sl, op0=op.mult, op1=op.add
            )
            lp = p_sc.tile([128, MC, BC], F32, tag="lp", name=f"lp_{w}_{s}")
            lu_sl = lu_t[:, :, s * BC : (s + 1) * BC]
            nc.vector.scalar_tensor_tensor(
                lp[:], psi[:], TH_PLUS, lu_sl, op0=op.mult, op1=op.add
            )
            ge_sl = ge_t[:, :, s * BC : (s + 1) * BC]
            nc.vector.tensor_tensor(ge_sl, mi[:], lp[:], op.is_le)
            nc.vector.tensor_tensor(mem[:], mi[:], ge_sl, op.mult)
            nc.vector.scalar_tensor_tensor(
                psi[:], psi[:], TH_DEC, ge_sl, op0=op.mult, op1=op.subtract
            )

        all_accs = [None] * NW
        for w in range(NW):
            ntw = WINS[w] * BC
            coff = OFF[w] * BC
            accs = [
                p_ps.tile([128, 512], F32, tag="acc", name=f"acc_{w}_{mc}")
                for mc in range(MC)
            ]
            all_accs[w] = accs
            lu_t = p_lu.tile([128, MC, 512], F16, tag="lu", name=f"lu_{w}")

            # ---- per-kc event schedule (rides on the GEMM phase) ----
            sched = {}

            def at(kc, fn):
                sched.setdefault(kc, []).append(fn)

            if w == 0:
                # consts land late in window 0 (first needed after GEMM_0);
                # keeps the scalar ring clear for the W1 chunk stream
                at(46, load_consts)
            # one full-window lu DMA on the scalar HWDGE ring; window 0's
            # waits until the W1 stream is nearly done (lu_0 is first read
            # by LIF1_0, after GEMM_0)
            at(46 if w == 0 else 2, lambda lu_t=lu_t, ntw=ntw, coff=coff:
               nc.scalar.dma_start(
                   lu_t[:, :, :ntw], lu1[:, :, coff : coff + ntw]))
            if w >= 2:
                # LIF2 of window w-2 rides early (c2 ready since window w-1)
                for s in range(WINS[w - 2]):
                    at(1 + s, lambda w=w, s=s: emit_lif2_step(w - 2, s))

            # ---------- FC1 GEMM for window w ----------
            for kc in range(KC):
                if w == 0:
                    # resident-W1 load: full chunks alternating between the
                    # two HWDGE rings so neither falls behind the MM stream;
                    # kc=0 lands its first 128 cols separately so MM #0's
                    # weights arrive with minimum latency
                    eng = nc.scalar if kc % 2 == 0 else nc.sync
                    if kc == 0:
                        eng.dma_start(w1_sb[:, 0, 0:128], w1t[0:128, 0:128])
                        eng.dma_start(w1_sb[:, 0, 128:1024], w1t[0:128, 128:1024])
                    else:
                        eng.dma_start(
                            w1_sb[:, kc, :], w1t[kc * 128 : (kc + 1) * 128, :]
                        )
                x_t = p_x.tile([128, 512], F16, tag="x", name=f"x_{w}_{kc}")
                nc.sync.dma_start(
                    x_t[:, :ntw], xt[kc * 128 : (kc + 1) * 128, coff : coff + ntw]
                )
                for fn in sched.get(kc, ()):
                    fn()
                for mc in range(MC):
                    nc.tensor.matmul(
                        accs[mc][:, :ntw],
                        w1_sb[:, kc, mc * 128 : (mc + 1) * 128],
                        x_t[:, :ntw],
                        start=(kc == 0),
                        stop=(kc == KC - 1),
                    )

            # ---------- psum -> sbuf, fused +b1 (split DVE/ACT) ----------
            # copies MUST precede emit_fc2_post on the DVE queue: ps2_{w-1}'s
            # ring slot frees via copy_w[0], and the fc2 subtract would
            # otherwise wait on it from ahead of it in the same FIFO
            cur1 = p_cur.tile([128, MC, 512], F32, tag="cur1", name=f"cur1_{w}")
            for mc in range(4):
                nc.vector.tensor_scalar_add(
                    cur1[:, mc, :ntw], accs[mc][:, :ntw], b1_sb[:, mc : mc + 1]
                )
            for mc in range(4, MC):
                nc.scalar.activation(
                    cur1[:, mc, :ntw],
                    accs[mc][:, :ntw],
                    AF.Identity,
                    bias=b1_sb[:, mc : mc + 1],
                    scale=1.0,
                )

            # FC2 of window w-1 right after GEMM_w (its psum ring slot is
            # freed by the first cur1 copy of window w, moments later)
            if w >= 1:
                emit_fc2_post(w - 1)

            # ---------- LIF1 recurrence for window w ----------
            ge_t = p_ge.tile([128, MC, 512], F16, tag="ge", name=f"ge_{w}")
            ge_tiles[w] = ge_t
            if w < NW - 1:
                for s in range(WINS[w]):
                    emit_rec1_step(w, s, cur1, lu_t, ge_t)
            else:
                # ---------- tail ----------
                # LIF1_4 with FC2_4 chunks pipelined one step behind (PE does
                # each chunk while DVE runs the next LIF1 step), then the
                # LIF2 chain for windows 3 and 4 (mem2 order: 3 before 4)
                c2_4 = p_c2.tile([A, 512], F32, tag="c2", name="c2_4")
                c2_tiles[4] = c2_4
                ps2_4 = p_ps.tile([128, 512], F32, tag="acc", name="ps2_4")
                for s in range(WINS[w]):
                    emit_rec1_step(w, s, cur1, lu_t, ge_t)
                    sl = slice(s * BC, (s + 1) * BC)
                    for k2 in range(MC):
                        nc.tensor.matmul(
                            ps2_4[:A, sl],
                            w2_sb[:, k2, :],
                            ge_t[:, k2, sl],
                            start=(k2 == 0),
                            stop=(k2 == MC - 1),
                        )
                    nc.vector.tensor_tensor(
                        c2_4[:, sl], rs2_sb[:, sl], ps2_4[:A, sl], op.subtract
                    )
                for s in range(WINS[NW - 2]):
                    emit_lif2_step(NW - 2, s)
                for s in range(WINS[w]):
                    emit_lif2_step(w, s)

        outf = p_st.tile([A, BC], F32, name="outf")
        nc.scalar.activation(outf[:], sum2[:], AF.Copy, bias=1.0, scale=-1.0 / T)
        nc.sync.dma_start(out[:], outf[:])

    nc.compile()
    return nc


def _host_prep(x, W1, b1, W2, b2, u1, u2):
    """Shard + lay out inputs for the 8 cores. Returns in_maps."""
    x16 = np.asarray(x, dtype=np.float16)
    W1 = np.asarray(W1, dtype=np.float32)
    b1 = np.asarray(b1, dtype=np.float32)
    W2 = np.asarray(W2, dtype=np.float32)
    b2 = np.asarray(b2, dtype=np.float32)

    # logit transform of the pre-drawn uniforms (in f64 for boundary accuracy);
    # the +1.5 absorbs the psi = 20*theta - 10 affine map
    with np.errstate(divide="ignore"):
        u1d = np.asarray(u1, dtype=np.float64)
        lu1f = (np.log(u1d / (1.0 - u1d)) + 1.5).astype(np.float16)
        u2d = np.asarray(u2, dtype=np.float64)
        lu2f = (np.log(u2d / (1.0 - u2d)) + 1.5).astype(np.float16)

    W1TP = np.zeros((D, HP), np.float16)
    W1TP[:, :H] = W1.T.astype(np.float16)
    w1t = np.ascontiguousarray(W1TP)

    b1p = np.zeros((HP,), np.float32)
    b1p[:H] = b1
    b1t = np.ascontiguousarray(b1p.reshape(MC, 128).T)  # [128, MC]

    W2TP = np.zeros((HP, A), np.float16)
    W2TP[:H, :] = W2.T.astype(np.float16)
    w2t = np.ascontiguousarray(W2TP.reshape(MC, 128, A).transpose(1, 0, 2))

    rs2 = (W2.astype(np.float64).sum(axis=1) + b2).astype(np.float32)  # [A]
    rs2b = np.ascontiguousarray(np.repeat(rs2[:, None], 512, axis=1))  # [A, 512]

    in_maps = []
    for c in range(NCORES):
        bs, be = c * BC, (c + 1) * BC
        # xt: [D, bt] with bt = t*BC + b
        xt_c = np.ascontiguousarray(x16[bs:be].transpose(2, 1, 0).reshape(D, BT))
        # lu1: [128, MC, bt]
        lu_c = np.full((T, BC, HP), LUPAD, np.float16)
        lu_c[:, :, :H] = lu1f[:, bs:be, :]
        lu_c = lu_c.transpose(2, 0, 1).reshape(HP, BT)  # [h, t*BC+b]
        lu_c = np.ascontiguousarray(lu_c.reshape(MC, 128, BT).transpose(1, 0, 2))
        # lu2: [A, T, BC]
        lu2_c = np.ascontiguousarray(lu2f[:, bs:be, :].transpose(2, 0, 1))
        in_maps.append(
            {
                "xt": xt_c,
                "w1t": w1t,
                "b1t": b1t,
                "lu1": lu_c,
                "lu2": lu2_c,
                "w2t": w2t,
                "rs2b": rs2b,
            }
        )
    return in_maps


def run(inputs, trace=False):
    """Build (cached), run on 8 cores, gather. Returns (out, BassKernelResults)."""
    from concourse.bass_utils import run_bass_kernel_spmd

    if "nc" not in _CACHE:
        _CACHE["nc"] = _build_graph()
    nc = _CACHE["nc"]
    in_maps = _host_prep(**inputs)
    res = run_bass_kernel_spmd(nc, in_maps, core_ids=list(range(NCORES)), trace=trace)
    # per-core output is [A, BC] -> transpose and stack to [B, A]
    out = np.concatenate(
        [res.results[c]["out"].T for c in range(NCORES)], axis=0
    )
    return np.ascontiguousarray(out, dtype=np.float32), res


def kernel(**inputs) -> np.ndarray:
    out, _ = run(inputs, trace=False)
    return out


# revision 31
# speedup vs baseline: 1.0258x; 1.0258x over previous
"""Trainium2 Bass kernel for AdaptiveStochasticSNN.

Model: x[B,T,D] -> FC1(D->H) -> StochasticAdaptiveLIF -> FC2(H->A)
       -> StochasticAdaptiveLIF -> mean spikes over T.   B,T,D,H,A = 256,64,6400,1000,4

Strategy (8 NeuronCores, data-parallel over batch, 32 batches/core):
- FC1 hoisted out of the time loop as one big GEMM x[bt,D] @ W1T[D,H] in fp16
  (full PE rate; fp16's 11-bit significand rounds identically to f32r/TF32 for
  this data range). fp16 storage halves HBM traffic and lets all of W1T
  (13.1MB) stay SBUF-resident, so the kernel is TensorE-bound, not DMA-bound.
- The bernoulli draw  u < sigmoid(mem - 1 - theta)  is monotone-transformed on
  the host to  logit(u) + 1 < mem - theta  (plain DVE compare, no sigmoid).
- LIF1 runs as fused scalar_tensor_tensor ops on VectorE with h on partitions;
  FC2 consumes the spike complement ge (= 1 - spk): cur2 = (rowsum(W2)+b2)
  - W2 @ ge, as [A, bt] so LIF2 runs on partitions 0..3 with no shuffle.
- 5 tapered windows of [16,16,16,12,4] timesteps pipeline GEMM(w) over
  LIF1(w-1) and FC2/LIF2(w-2). The taper keeps the serial DVE tail short.
- Two HWDGE DMA rings: the sync ring streams x chunks; the scalar ring takes
  the W1-resident load, per-window logit-u tiles, and constants.
"""

import sys

sys.path.insert(0, "/opt/trn_rl_repo")

import numpy as np

# ---- problem dims (hardcoded; kernel.py must be self-contained) ----
B, T, D, H, A = 256, 64, 6400, 1000, 4
HP = 1024          # H padded to 8*128
NCORES = 8
BC = B // NCORES   # 32 batches per core
BT = BC * T        # 2048 bt-columns per core, ordered bt = t*BC + b
KC = D // 128      # 50 contraction chunks
MC = HP // 128     # 8 h-chunks
WINS = [16, 16, 16, 12, 4]   # timesteps per window (sum = T)
NW = len(WINS)
OFF = [sum(WINS[:i]) for i in range(NW)]
BETA = 0.9
TH_DEC = 0.9
TH_PLUS = 0.05
LUPAD = np.float16(60000.0)  # threshold for padded h lanes: never spike

_CACHE = {}


def _build_graph():
    import concourse.bass as bass
    import concourse.tile as tile
    from concourse import bacc, mybir
    from concourse.alu_op_type import AluOpType as op
    from contextlib import ExitStack

    F32 = mybir.dt.float32
    F16 = mybir.dt.float16
    AF = mybir.ActivationFunctionType

    nc = bacc.Bacc("TRN2", target_bir_lowering=False, debug=False, num_devices=NCORES)

    xt = nc.declare_dram_parameter("xt", [D, BT], F16, isOutput=False)
    w1t = nc.declare_dram_parameter("w1t", [D, HP], F16, isOutput=False)
    b1t = nc.declare_dram_parameter("b1t", [128, MC], F32, isOutput=False)
    lu1 = nc.declare_dram_parameter("lu1", [128, MC, BT], F16, isOutput=False)
    lu2 = nc.declare_dram_parameter("lu2", [A, T, BC], F16, isOutput=False)
    w2t = nc.declare_dram_parameter("w2t", [128, MC, A], F16, isOutput=False)
    rs2b = nc.declare_dram_parameter("rs2b", [A, 512], F32, isOutput=False)
    out = nc.declare_dram_parameter("out", [A, BC], F32, isOutput=True)

    with tile.TileContext(nc) as tc, ExitStack() as ctx:
        p_w1 = ctx.enter_context(tc.tile_pool(name="w1p", bufs=1))
        p_x = ctx.enter_context(tc.tile_pool(name="xp", bufs=6))
        p_lu = ctx.enter_context(tc.tile_pool(name="lup", bufs=2))
        p_cur = ctx.enter_context(tc.tile_pool(name="curp", bufs=1))
        p_ge = ctx.enter_context(tc.tile_pool(name="gep", bufs=2))
        p_sc = ctx.enter_context(tc.tile_pool(name="scp", bufs=2))
        p_c2 = ctx.enter_context(tc.tile_pool(name="c2p", bufs=2))
        p_st = ctx.enter_context(tc.tile_pool(name="stp", bufs=1))
        p_ps = ctx.enter_context(
            tc.tile_pool(name="psp", bufs=8, space=bass.MemorySpace.PSUM)
        )

        # ---- persistent tiles ----
        b1_sb = p_st.tile([128, MC], F32, name="b1_sb")
        w2_sb = p_st.tile([128, MC, A], F16, name="w2_sb")
        rs2_sb = p_st.tile([A, 512], F32, name="rs2_sb")
        lu2_sb = p_st.tile([A, T, BC], F16, name="lu2_sb")

        # W1T fully resident: [128, KC, HP] fp16 = 100KB/partition
        w1_sb = p_w1.tile([128, KC, HP], F16, name="w1_sb")

        # theta is tracked as psi = 20*theta - 10, turning the update into a
        # single fused op  psi' = 0.9*psi - ge  (the compare absorbs the
        # affine map via a host-side +1.5 on the logit)
        mem = p_st.tile([128, MC, BC], F32, name="mem")
        nc.gpsimd.memset(mem[:], 0.0)
        psi = p_st.tile([128, MC, BC], F32, name="psi")
        nc.gpsimd.memset(psi[:], -10.0)
        mem2 = p_st.tile([A, BC], F32, name="mem2")
        nc.gpsimd.memset(mem2[:], 0.0)
        psi2 = p_st.tile([A, BC], F32, name="psi2")
        nc.gpsimd.memset(psi2[:], -10.0)
        sum2 = p_st.tile([A, BC], F32, name="sum2")
        nc.gpsimd.memset(sum2[:], 0.0)

        ge_tiles = [None] * NW
        c2_tiles = [None] * NW

        def load_consts():
            # scalar-ring so the sync ring stays dedicated to the x stream
            nc.scalar.dma_start(b1_sb[:], b1t[:])
            nc.scalar.dma_start(w2_sb[:], w2t[:])
            nc.scalar.dma_start(rs2_sb[:], rs2b[:])
            nc.scalar.dma_start(lu2_sb[:], lu2[:])

        def emit_fc2_post(w):
            """FC2 for window w as one full-width chunk, own psum tile
            (emitted right after GEMM of window w+1; ring slot is free)."""
            ntw = WINS[w] * BC
            ps2 = p_ps.tile([128, 512], F32, tag="acc", name=f"ps2_{w}")
            c2 = p_c2.tile([A, 512], F32, tag="c2", name=f"c2_{w}")
            c2_tiles[w] = c2
            ge_t = ge_tiles[w]
            for k2 in range(MC):
                nc.tensor.matmul(
                    ps2[:A, :ntw],
                    w2_sb[:, k2, :],
                    ge_t[:, k2, :ntw],
                    start=(k2 == 0),
                    stop=(k2 == MC - 1),
                )
            nc.vector.tensor_tensor(
                c2[:, :ntw], rs2_sb[:, :ntw], ps2[:A, :ntw], op.subtract
            )

        def emit_lif2_step(w, s):
            t = OFF[w] + s
            eng = nc.vector
            cur2 = c2_tiles[w][:, s * BC : (s + 1) * BC]
            m2i = p_sc.tile([A, BC], F32, tag="m2i", name=f"m2i_{t}")
            eng.scalar_tensor_tensor(
                m2i[:], mem2[:], BETA, cur2, op0=op.mult, op1=op.add
            )
            lp2 = p_sc.tile([A, BC], F32, tag="lp2", name=f"lp2_{t}")
            eng.scalar_tensor_tensor(
                lp2[:], psi2[:], TH_PLUS, lu2_sb[:, t, :], op0=op.mult, op1=op.add
            )
            ge2 = p_sc.tile([A, BC], F32, tag="ge2", name=f"ge2_{t}")
            eng.tensor_tensor(ge2[:], m2i[:], lp2[:], op.is_le)
            eng.tensor_tensor(mem2[:], m2i[:], ge2[:], op.mult)
            eng.scalar_tensor_tensor(
                psi2[:], psi2[:], TH_DEC, ge2[:], op0=op.mult, op1=op.subtract
            )
            eng.tensor_tensor(sum2[:], sum2[:], ge2[:], op.add)

        def emit_rec1_step(w, s, cur1, lu_t, ge_t):
            c_sl = cur1[:, :, s * BC : (s + 1) * BC]
            mi = p_sc.tile([128, MC, BC], F32, tag="mi", name=f"mi_{w}_{s}")
            nc.vector.scalar_tensor_tensor(
                mi[:], mem[:], BETA, c_sl, op0=op.mult, op1=op.add
            )
            lp = p_sc.tile([128, MC, BC], F32, tag="lp", name=f"lp_{w}_{s}")
            lu_sl = lu_t[:, :, s * BC : (s + 1) * BC]
            nc.vector.scalar_tensor_tensor(
                lp[:], psi[:], TH_PLUS, lu_sl, op0=op.mult, op1=op.add
            )
            ge_sl = ge_t[:, :, s * BC : (s + 1) * BC]
            nc.vector.tensor_tensor(ge_sl, mi[:], lp[:], op.is_le)
            nc.vector.tensor_tensor(mem[:], mi[:], ge_sl, op.mult)
            nc.vector.scalar_tensor_tensor(
                psi[:], psi[:], TH_DEC, ge_sl, op0=op.mult, op1=op.subtract
            )

        all_accs = [None] * NW
        for w in range(NW):
            ntw = WINS[w] * BC
            coff = OFF[w] * BC
            accs = [
                p_ps.tile([128, 512], F32, tag="acc", name=f"acc_{w}_{mc}")
                for mc in range(MC)
            ]
            all_accs[w] = accs
            lu_t = p_lu.tile([128, MC, 512], F16, tag="lu", name=f"lu_{w}")

            # ---- per-kc event schedule (rides on the GEMM phase) ----
            sched = {}

            def at(kc, fn):
                sched.setdefault(kc, []).append(fn)

            if w == 0:
                # consts land late in window 0 (first needed after GEMM_0);
                # keeps the scalar ring clear for the W1 chunk stream
                at(46, load_consts)
            # one full-window lu DMA on the scalar HWDGE ring; window 0's
            # waits until the W1 stream is nearly done (lu_0 is first read
            # by LIF1_0, after GEMM_0)
            at(46 if w == 0 else 2, lambda lu_t=lu_t, ntw=ntw, coff=coff:
               nc.scalar.dma_start(
                   lu_t[:, :, :ntw], lu1[:, :, coff : coff + ntw]))
            if w >= 2:
                # LIF2 of window w-2 rides early (c2 ready since window w-1)
                for s in range(WINS[w - 2]):
                    at(1 + s, lambda w=w, s=s: emit_lif2_step(w - 2, s))

            # ---------- FC1 GEMM for window w ----------
            for kc in range(KC):
                if w == 0:
                    # resident-W1 load: one full chunk per kc on the scalar
                    # ring (the sync ring carries only the x stream)
                    nc.scalar.dma_start(
                        w1_sb[:, kc, :], w1t[kc * 128 : (kc + 1) * 128, :]
                    )
                x_t = p_x.tile([128, 512], F16, tag="x", name=f"x_{w}_{kc}")
                nc.sync.dma_start(
                    x_t[:, :ntw], xt[kc * 128 : (kc + 1) * 128, coff : coff + ntw]
                )
                for fn in sched.get(kc, ()):
                    fn()
                for mc in range(MC):
                    nc.tensor.matmul(
                        accs[mc][:, :ntw],
                        w1_sb[:, kc, mc * 128 : (mc + 1) * 128],
                        x_t[:, :ntw],
                        start=(kc == 0),
                        stop=(kc == KC - 1),
                    )

            # ---------- psum -> sbuf, fused +b1 (split DVE/ACT) ----------
            # copies MUST precede emit_fc2_post on the DVE queue: ps2_{w-1}'s
            # ring slot frees via copy_w[0], and the fc2 subtract would
            # otherwise wait on it from ahead of it in the same FIFO
            cur1 = p_cur.tile([128, MC, 512], F32, tag="cur1", name=f"cur1_{w}")
            for mc in range(4):
                nc.vector.tensor_scalar_add(
                    cur1[:, mc, :ntw], accs[mc][:, :ntw], b1_sb[:, mc : mc + 1]
                )
            for mc in range(4, MC):
                nc.scalar.activation(
                    cur1[:, mc, :ntw],
                    accs[mc][:, :ntw],
                    AF.Identity,
                    bias=b1_sb[:, mc : mc + 1],
                    scale=1.0,
                )

            # FC2 of window w-1 right after GEMM_w (its psum ring slot is
            # freed by the first cur1 copy of window w, moments later)
            if w >= 1:
                emit_fc2_post(w - 1)

            # ---------- LIF1 recurrence for window w ----------
            ge_t = p_ge.tile([128, MC, 512], F16, tag="ge", name=f"ge_{w}")
            ge_tiles[w] = ge_t
            if w < NW - 1:
                for s in range(WINS[w]):
                    emit_rec1_step(w, s, cur1, lu_t, ge_t)
            else:
                # ---------- tail ----------
                # LIF1_4 with FC2_4 chunks pipelined one step behind (PE does
                # each chunk while DVE runs the next LIF1 step), then the
                # LIF2 chain for windows 3 and 4 (mem2 order: 3 before 4)
                c2_4 = p_c2.tile([A, 512], F32, tag="c2", name="c2_4")
                c2_tiles[4] = c2_4
                ps2_4 = p_ps.tile([128, 512], F32, tag="acc", name="ps2_4")
                for s in range(WINS[w]):
                    emit_rec1_step(w, s, cur1, lu_t, ge_t)
                    sl = slice(s * BC, (s + 1) * BC)
                    for k2 in range(MC):
                        nc.tensor.matmul(
                            ps2_4[:A, sl],
                            w2_sb[:, k2, :],
                            ge_t[:, k2, sl],
                            start=(k2 == 0),
                            stop=(k2 == MC - 1),
                        )
                    nc.vector.tensor_tensor(
                        c2_4[:, sl], rs2_sb[:, sl], ps2_4[:A, sl], op.subtract
                    )
                for s in range(WINS[NW - 2]):
                    emit_lif2_step(NW - 2, s)
                for s in range(WINS[w]):
                    emit_lif2_step(w, s)

        outf = p_st.tile([A, BC], F32, name="outf")
        nc.scalar.activation(outf[:], sum2[:], AF.Copy, bias=1.0, scale=-1.0 / T)
        nc.sync.dma_start(out[:], outf[:])

    nc.compile()
    return nc


def _host_prep(x, W1, b1, W2, b2, u1, u2):
    """Shard + lay out inputs for the 8 cores. Returns in_maps."""
    x16 = np.asarray(x, dtype=np.float16)
    W1 = np.asarray(W1, dtype=np.float32)
    b1 = np.asarray(b1, dtype=np.float32)
    W2 = np.asarray(W2, dtype=np.float32)
    b2 = np.asarray(b2, dtype=np.float32)

    # logit transform of the pre-drawn uniforms (in f64 for boundary accuracy);
    # the +1.5 absorbs the psi = 20*theta - 10 affine map
    with np.errstate(divide="ignore"):
        u1d = np.asarray(u1, dtype=np.float64)
        lu1f = (np.log(u1d / (1.0 - u1d)) + 1.5).astype(np.float16)
        u2d = np.asarray(u2, dtype=np.float64)
        lu2f = (np.log(u2d / (1.0 - u2d)) + 1.5).astype(np.float16)

    W1TP = np.zeros((D, HP), np.float16)
    W1TP[:, :H] = W1.T.astype(np.float16)
    w1t = np.ascontiguousarray(W1TP)

    b1p = np.zeros((HP,), np.float32)
    b1p[:H] = b1
    b1t = np.ascontiguousarray(b1p.reshape(MC, 128).T)  # [128, MC]

    W2TP = np.zeros((HP, A), np.float16)
    W2TP[:H, :] = W2.T.astype(np.float16)
    w2t = np.ascontiguousarray(W2TP.reshape(MC, 128, A).transpose(1, 0, 2))

    rs2 = (W2.astype(np.float64).sum(axis=1) + b2).astype(np.float32)  # [A]
    rs2b = np.ascontiguousarray(np.repeat(rs2[:, None], 512, axis=1))  # [A, 512]

    in_maps = []
    for c in range(NCORES):
        bs, be = c * BC, (c + 1) * BC
        # xt: [D, bt] with bt = t*BC + b
        xt_c = np.ascontiguousarray(x16[bs:be].transpose(2, 1, 0).reshape(D, BT))
        # lu1: [128, MC, bt]
        lu_c = np.full((T, BC, HP), LUPAD, np.float16)
        lu_c[:, :, :H] = lu1f[:, bs:be, :]
        lu_c = lu_c.transpose(2, 0, 1).reshape(HP, BT)  # [h, t*BC+b]
        lu_c = np.ascontiguousarray(lu_c.reshape(MC, 128, BT).transpose(1, 0, 2))
        # lu2: [A, T, BC]
        lu2_c = np.ascontiguousarray(lu2f[:, bs:be, :].transpose(2, 0, 1))
        in_maps.append(
            {
                "xt": xt_c,
                "w1t": w1t,
                "b1t": b1t,
                "lu1": lu_c,
                "lu2": lu2_c,
                "w2t": w2t,
                "rs2b": rs2b,
            }
        )
    return in_maps


def run(inputs, trace=False):
    """Build (cached), run on 8 cores, gather. Returns (out, BassKernelResults)."""
    from concourse.bass_utils import run_bass_kernel_spmd

    if "nc" not in _CACHE:
        _CACHE["nc"] = _build_graph()
    nc = _CACHE["nc"]
    in_maps = _host_prep(**inputs)
    res = run_bass_kernel_spmd(nc, in_maps, core_ids=list(range(NCORES)), trace=trace)
    # per-core output is [A, BC] -> transpose and stack to [B, A]
    out = np.concatenate(
        [res.results[c]["out"].T for c in range(NCORES)], axis=0
    )
    return np.ascontiguousarray(out, dtype=np.float32), res


def kernel(**inputs) -> np.ndarray:
    out, _ = run(inputs, trace=False)
    return out
